# revision 26
# baseline (speedup 1.0000x reference)
"""MoE FFN (top-2 of 8 experts) Trainium2 kernel.

Strategy (expert-parallel across 8 NeuronCores):
  - Host computes the (tiny) router: logits = x@Wg, softmax, top-2,
    renormalized combine weights.  Tokens are gathered per expert on the
    host ("all-to-all dispatch" done at sharding time), transposed to
    [H, C] so both FFN GEMMs run with natural weight layouts on device.
  - Core e runs the FFN for expert e over its C_pad gathered tokens.
    All of W1 and W2 are converted to bf16 and kept RESIDENT in SBUF
    (128 KiB/partition), so each token chunk makes a single pass:
        hT = gelu_tanh(W1.T-tiles @ xT)       # [F, C] via 32 f-tiles
        Y  = sum_f hT-tiles.T @ W2-tiles      # [C, H], full F in PSUM
    and Y is written once (bf16).  The tensor engine runs a single
    gapless stream of back-to-back N<=512 matmuls (bf16 = full rate,
    same as fp32r, half the bytes); HBM traffic is ~26MB/core vs the
    baseline's 109MB.
  - x is staged chunk-major so every chunk load is one contiguous
    8KB/partition DMA (the strided [KH, c_pad] slice was ~10x slower
    and dominated the kernel head).
  - Host applies combine weights + b2 and scatter-adds back ("combine").

The kernel is compiled once per (C_pad, chunk-structure, biases-zero)
configuration and cached in-process.
"""

import os
import sys
import numpy as np

for _p in ("/opt/trn_rl_repo", "/root/.axon_site/_ro/trn_rl_repo"):
    if _p not in sys.path and os.path.isdir(_p):
        sys.path.append(_p)

import concourse.bacc as bacc  # noqa: E402
import concourse.tile as tile  # noqa: E402
from concourse import mybir  # noqa: E402
from concourse.bass_utils import run_bass_kernel_spmd  # noqa: E402

# Problem shapes (hardcoded per spec)
B, S, H, F, E = 4, 2048, 1024, 4096, 8
T = B * S
TOP_K = 2
N_CORES = 8
P = 128
KH = H // P          # 8  H-contraction subtiles
FT = F // P          # 32 f-tiles total

F32 = mybir.dt.float32
BF16 = mybir.dt.bfloat16

# W1 streamed in f-tile slices; small lead slices let GEMM1 start early.
W1_SLICES = (1, 1, 2, 4, 4, 4, 4, 4, 4, 4)
W2_SLICES = (8, 8, 8, 8)

_CACHE: dict = {}
LAST_RESULT = None  # BassKernelResults of the most recent run (for test.py)


def _chunks_for(c_pad: int) -> tuple:
    """Token chunks: 512s with an optional exact-size tail."""
    out = [512] * (c_pad // 512)
    if c_pad % 512:
        out.append(c_pad % 512)
    return tuple(out)


def _build(c_pad: int, chunks: tuple, use_b1: bool):
    n512 = sum(1 for c in chunks if c == 512)
    tail = chunks[-1] if chunks[-1] != 512 else None
    nrows = -(-c_pad // P)  # output t-tile rows (last may be partial)

    nc = bacc.Bacc(
        "TRN2",
        target_bir_lowering=False,
        debug=False,
        enable_asserts=False,
        num_devices=N_CORES,
    )

    # x staged chunk-major: each chunk is contiguous per partition.
    xda = nc.dram_tensor("xda", [P, n512, KH, 512], BF16, kind="ExternalInput").ap()
    if tail:
        xdb = nc.dram_tensor("xdb", [P, KH, tail], BF16, kind="ExternalInput").ap()
    w1d = nc.dram_tensor("w1d", [P, FT, KH, P], BF16, kind="ExternalInput").ap()
    w2d = nc.dram_tensor("w2d", [P, FT, H], BF16, kind="ExternalInput").ap()
    if use_b1:
        b1d = nc.dram_tensor("b1d", [P, FT], F32, kind="ExternalInput").ap()
    yd = nc.dram_tensor("yd", [P, nrows, H], BF16, kind="ExternalOutput").ap()

    gelu = mybir.ActivationFunctionType.Gelu_apprx_tanh

    with tile.TileContext(nc) as tc:
        with (
            tc.tile_pool(name="w1p", bufs=1) as w1p,
            tc.tile_pool(name="w2p", bufs=1) as w2p,
            tc.tile_pool(name="xp", bufs=2) as xp,
            tc.tile_pool(name="hp", bufs=1) as hp,
            tc.tile_pool(name="op", bufs=4) as op,
            tc.tile_pool(name="bp", bufs=1) as bp,
            tc.tile_pool(name="wup", bufs=1) as wup,
            tc.tile_pool(name="ps1", bufs=3, space="PSUM") as ps1,
            tc.tile_pool(name="ps2", bufs=4, space="PSUM") as ps2,
            tc.tile_pool(name="wupp", bufs=1, space="PSUM") as wupp,
        ):
            if use_b1:
                b1t = bp.tile([P, FT], F32)
                nc.sync.dma_start(b1t[:], b1d[:])

            # HAM warmup: the PE clock-gate sits at 1.2 GHz until it sees
            # ~3.4us of sustained matmul activity.  The PE is idle anyway
            # while the first weights/x stream in (~0-11us), so burn that
            # window on junk N=64 matmuls into a scratch PSUM bank; the
            # real stream then starts at the full 2.4 GHz.
            wub = wup.tile([P, P], BF16)
            nc.gpsimd.memset(wub[:], 0)
            wups = wupp.tile([P, 64], F32)
            for _ in range(90):
                nc.tensor.matmul(wups[:], wub[:], wub[:, :64], start=True, stop=True)

            # Chunk-0 x rides at the FRONT of the gpsimd queue (it gates
            # the very first matmul; anywhere behind or beside the 16MB
            # weight stream it lands ~10us late because all DMA queues
            # share the 16 SDMA engines).
            xt0 = xp.tile([P, KH, chunks[0]], BF16, tag="xt")
            if chunks[0] == 512:
                nc.gpsimd.dma_start(xt0[:, :4], xda[:, 0, :4])
                nc.gpsimd.dma_start(xt0[:, 4:], xda[:, 0, 4:])
            else:
                nc.gpsimd.dma_start(xt0[:], xdb[:])

            # Weights resident in SBUF for the whole kernel.  The two lead
            # W1 slices ride the sync HWDGE ring right after chunk-0 x
            # (lowest latency); the bulk rides the gpsimd SWDGE path: the
            # sync ring carries x/y, and the scalar engine must stay free
            # for the GEMM1 activations (DMA issue ops block the issuing
            # engine on HWDGE FIFO space -- parking 18 weight DMAs on the
            # scalar queue delayed the first gelu to ~42us and stalled the
            # PE cold).  W1 first -- GEMM1 needs its lead slices
            # immediately; W2 isn't read until GEMM1 of chunk 0 finishes
            # (~50us in).
            w1t = w1p.tile([P, FT, KH, P], BF16)
            f0 = 0
            for si, g in enumerate(W1_SLICES):
                eng = nc.sync if si < 3 else nc.gpsimd
                eng.dma_start(w1t[:, f0 : f0 + g], w1d[:, f0 : f0 + g])
                f0 += g
            w2t = w2p.tile([P, FT, H], BF16)
            f0 = 0
            for g in W2_SLICES:
                nc.gpsimd.dma_start(w2t[:, f0 : f0 + g], w2d[:, f0 : f0 + g])
                f0 += g

            coff = 0
            for ci, nt in enumerate(chunks):
                if ci == 0:
                    xt = xt0
                else:
                    # later chunks have ~2 chunk-times of slack; ride the
                    # gpsimd queue BEHIND the weight bulk so they don't
                    # steal head HBM bandwidth from the W1 ramp.
                    xt = xp.tile([P, KH, nt], BF16, tag="xt")
                    if nt == 512:
                        nc.gpsimd.dma_start(xt[:], xda[:, ci])
                    else:
                        nc.gpsimd.dma_start(xt[:], xdb[:])

                # GEMM1: hT[f, :] = gelu(sum_k W1[k, f-tile].T @ xT[k, :])
                hq = hp.tile([P, FT, nt], BF16, tag="hq")
                for f in range(FT):
                    pt1 = ps1.tile([P, nt], F32, tag="pt1")
                    for k in range(KH):
                        nc.tensor.matmul(
                            pt1[:],
                            w1t[:, f, k, :],
                            xt[:, k, :],
                            start=(k == 0),
                            stop=(k == KH - 1),
                        )
                    bias = b1t[:, f : f + 1] if use_b1 else 0.0
                    nc.scalar.activation(hq[:, f, :], pt1[:], gelu, bias=bias)
                    if ci == 0 and 1 <= f <= 6:
                        # keep the PE busy through the HBM-bound W1 ramp
                        # (waits here otherwise re-throttle the HAM gate)
                        for _ in range(8):
                            nc.tensor.matmul(
                                wups[:], wub[:], wub[:, :64],
                                start=True, stop=True,
                            )

                # GEMM2 (full F accumulation in PSUM):
                # Y[t-tile, hh] = sum_k2 hT[k2, t-tile].T @ W2[k2, hh]
                for t in range(-(-nt // P)):
                    tp = min(P, nt - t * P)  # partial tail t-tile
                    trow = coff // P + t
                    for hh in range(2):
                        pt2 = ps2.tile([P, 512], F32, tag="pt2")
                        for k2 in range(FT):
                            nc.tensor.matmul(
                                pt2[:tp],
                                hq[:, k2, t * P : t * P + tp],
                                w2t[:, k2, hh * 512 : (hh + 1) * 512],
                                start=(k2 == 0),
                                stop=(k2 == FT - 1),
                            )
                        ot = op.tile([P, 512], BF16, tag="ot")
                        nc.vector.tensor_copy(ot[:tp], pt2[:tp])
                        nc.sync.dma_start(
                            yd[:tp, trow, hh * 512 : (hh + 1) * 512], ot[:tp]
                        )
                coff += nt

    nc.compile()
    return nc


def _gelu_tanh(v):
    # jax.nn.gelu(approximate=True): 0.5x(1+tanh(sqrt(2/pi)(x+0.044715x^3)))
    return 0.5 * v * (1.0 + np.tanh(0.7978845608028654 * (v + 0.044715 * v**3)))


def _route(x2d, Wg):
    """Replicates reference router: softmax -> top-2 -> renormalize."""
    logits = x2d @ Wg  # [T, E] fp32
    m = logits.max(axis=-1, keepdims=True)
    p = np.exp(logits - m, dtype=np.float32)
    p /= p.sum(axis=-1, keepdims=True)
    # jax.lax.top_k: values descending, ties broken by lower index.
    order = np.argsort(-p, axis=-1, kind="stable")
    top_i = order[:, :TOP_K]  # [T, 2]
    top_p = np.take_along_axis(p, top_i, axis=-1)
    top_p = top_p / top_p.sum(axis=-1, keepdims=True)
    return top_i, top_p


def kernel(x, Wg, W1, b1, W2, b2):
    global LAST_RESULT
    x = np.ascontiguousarray(np.asarray(x, dtype=np.float32))
    Wg = np.ascontiguousarray(np.asarray(Wg, dtype=np.float32))
    W1 = np.ascontiguousarray(np.asarray(W1, dtype=np.float32))
    b1 = np.ascontiguousarray(np.asarray(b1, dtype=np.float32))
    W2 = np.ascontiguousarray(np.asarray(W2, dtype=np.float32))
    b2 = np.ascontiguousarray(np.asarray(b2, dtype=np.float32))

    x2d = x.reshape(T, H)
    top_i, top_p = _route(x2d, Wg)

    rows = [None] * E
    gval = [None] * E
    for e in range(E):
        r, slot = np.nonzero(top_i == e)
        rows[e] = r
        gval[e] = top_p[r, slot]

    # Expert capacity (factor 1.0): each core computes at most T*K/E =
    # 2048 token slots -- the perfectly balanced load, which keeps every
    # device chunk at the efficient 512 width and every GEMM2 t-tile at
    # full M=128.  The few overflow tokens of over-subscribed experts
    # (~1.8% of assignments for this routing) are evaluated in fp32
    # during the host-side combine below, exactly like the router and
    # gate application already are.
    cap = T * TOP_K // E
    c_max = max(len(r) for r in rows)
    c_pad = max(512, min(c_max, cap))
    chunks = _chunks_for(c_pad)
    n512 = sum(1 for c in chunks if c == 512)
    tail = chunks[-1] if chunks[-1] != 512 else None
    nrows = -(-c_pad // P)
    use_b1 = bool(np.any(b1))

    key = (c_pad, chunks, use_b1)
    if key not in _CACHE:
        _CACHE[key] = _build(c_pad, chunks, use_b1)
    nc = _CACHE[key]

    np_bf16 = mybir.dt.np(BF16)
    in_maps = []
    for e in range(E):
        cd = min(len(rows[e]), c_pad)
        xt = np.zeros((H, c_pad), np.float32)
        xt[:, :cd] = x2d[rows[e][:cd]].T
        # [P, KH, c_pad] view, then chunk-major repack
        xpkh = xt.reshape(KH, P, c_pad).transpose(1, 0, 2).astype(np_bf16)
        xa = np.ascontiguousarray(
            xpkh[:, :, : n512 * 512].reshape(P, KH, n512, 512).transpose(0, 2, 1, 3)
        )
        m = {
            "xda": xa,
            "w1d": np.ascontiguousarray(
                W1[e].reshape(KH, P, FT, P).transpose(1, 2, 0, 3).astype(np_bf16)
            ),
            "w2d": np.ascontiguousarray(
                W2[e].reshape(FT, P, H).transpose(1, 0, 2).astype(np_bf16)
            ),
        }
        if tail:
            m["xdb"] = np.ascontiguousarray(xpkh[:, :, n512 * 512 :])
        if use_b1:
            m["b1d"] = np.ascontiguousarray(b1[e].reshape(FT, P).T)
        in_maps.append(m)

    trace = os.environ.get("KERNEL_TRACE", "") == "1"
    res = run_bass_kernel_spmd(
        nc,
        in_maps,
        core_ids=list(range(N_CORES)),
        trace=trace,
        trace_cores=[0] if trace else None,
    )
    LAST_RESULT = res

    out = np.zeros((T, H), np.float32)
    for e in range(E):
        cd = min(len(rows[e]), c_pad)
        yt = res.results[e]["yd"].astype(np.float32)  # [P, nrows, H]
        y = yt.transpose(1, 0, 2).reshape(nrows * P, H)[:cd]
        out[rows[e][:cd]] += gval[e][:cd, None] * (y + b2[e][None, :])
        if len(rows[e]) > cd:  # capacity overflow: fp32 on host
            ro = rows[e][cd:]
            ho = _gelu_tanh(x2d[ro] @ W1[e] + b1[e][None, :])
            yo = ho @ W2[e] + b2[e][None, :]
            out[ro] += gval[e][cd:, None] * yo

    return out.reshape(B, S, H)


# revision 27
# speedup vs baseline: 1.0004x; 1.0004x over previous
"""MoE FFN (top-2 of 8 experts) Trainium2 kernel.

Strategy (expert-parallel across 8 NeuronCores):
  - Host computes the (tiny) router: logits = x@Wg, softmax, top-2,
    renormalized combine weights.  Tokens are gathered per expert on the
    host ("all-to-all dispatch" done at sharding time), transposed to
    [H, C] so both FFN GEMMs run with natural weight layouts on device.
  - Core e runs the FFN for expert e over its C_pad gathered tokens.
    All of W1 and W2 are converted to bf16 and kept RESIDENT in SBUF
    (128 KiB/partition), so each token chunk makes a single pass:
        hT = gelu_tanh(W1.T-tiles @ xT)       # [F, C] via 32 f-tiles
        Y  = sum_f hT-tiles.T @ W2-tiles      # [C, H], full F in PSUM
    and Y is written once (bf16).  The tensor engine runs a single
    gapless stream of back-to-back N<=512 matmuls (bf16 = full rate,
    same as fp32r, half the bytes); HBM traffic is ~26MB/core vs the
    baseline's 109MB.
  - x is staged chunk-major so every chunk load is one contiguous
    8KB/partition DMA (the strided [KH, c_pad] slice was ~10x slower
    and dominated the kernel head).
  - Host applies combine weights + b2 and scatter-adds back ("combine").

The kernel is compiled once per (C_pad, chunk-structure, biases-zero)
configuration and cached in-process.
"""

import os
import sys
import numpy as np

for _p in ("/opt/trn_rl_repo", "/root/.axon_site/_ro/trn_rl_repo"):
    if _p not in sys.path and os.path.isdir(_p):
        sys.path.append(_p)

import concourse.bacc as bacc  # noqa: E402
import concourse.tile as tile  # noqa: E402
from concourse import mybir  # noqa: E402
from concourse.bass_utils import run_bass_kernel_spmd  # noqa: E402

# Problem shapes (hardcoded per spec)
B, S, H, F, E = 4, 2048, 1024, 4096, 8
T = B * S
TOP_K = 2
N_CORES = 8
P = 128
KH = H // P          # 8  H-contraction subtiles
FT = F // P          # 32 f-tiles total

F32 = mybir.dt.float32
BF16 = mybir.dt.bfloat16

# W1 streamed in f-tile slices; small lead slices let GEMM1 start early.
W1_SLICES = (1, 1, 2, 4, 4, 4, 4, 4, 4, 4)
W2_SLICES = (8, 8, 8, 8)

_CACHE: dict = {}
LAST_RESULT = None  # BassKernelResults of the most recent run (for test.py)


def _chunks_for(c_pad: int) -> tuple:
    """Token chunks: 512s with an optional exact-size tail."""
    out = [512] * (c_pad // 512)
    if c_pad % 512:
        out.append(c_pad % 512)
    return tuple(out)


def _build(c_pad: int, chunks: tuple, use_b1: bool):
    n512 = sum(1 for c in chunks if c == 512)
    tail = chunks[-1] if chunks[-1] != 512 else None
    nrows = -(-c_pad // P)  # output t-tile rows (last may be partial)

    nc = bacc.Bacc(
        "TRN2",
        target_bir_lowering=False,
        debug=False,
        enable_asserts=False,
        num_devices=N_CORES,
    )

    # x staged chunk-major: each chunk is contiguous per partition.
    xda = nc.dram_tensor("xda", [P, n512, KH, 512], BF16, kind="ExternalInput").ap()
    if tail:
        xdb = nc.dram_tensor("xdb", [P, KH, tail], BF16, kind="ExternalInput").ap()
    w1d = nc.dram_tensor("w1d", [P, FT, KH, P], BF16, kind="ExternalInput").ap()
    w2d = nc.dram_tensor("w2d", [P, FT, H], BF16, kind="ExternalInput").ap()
    if use_b1:
        b1d = nc.dram_tensor("b1d", [P, FT], F32, kind="ExternalInput").ap()
    yd = nc.dram_tensor("yd", [P, nrows, H], BF16, kind="ExternalOutput").ap()

    gelu = mybir.ActivationFunctionType.Gelu_apprx_tanh

    with tile.TileContext(nc) as tc:
        with (
            tc.tile_pool(name="w1p", bufs=1) as w1p,
            tc.tile_pool(name="w2p", bufs=1) as w2p,
            tc.tile_pool(name="xp", bufs=2) as xp,
            tc.tile_pool(name="hp", bufs=1) as hp,
            tc.tile_pool(name="op", bufs=4) as op,
            tc.tile_pool(name="bp", bufs=1) as bp,
            tc.tile_pool(name="wup", bufs=1) as wup,
            tc.tile_pool(name="ps1", bufs=3, space="PSUM") as ps1,
            tc.tile_pool(name="ps2", bufs=4, space="PSUM") as ps2,
            tc.tile_pool(name="wupp", bufs=1, space="PSUM") as wupp,
        ):
            if use_b1:
                b1t = bp.tile([P, FT], F32)
                nc.sync.dma_start(b1t[:], b1d[:])

            # HAM warmup: the PE clock-gate sits at 1.2 GHz until it sees
            # ~3.4us of sustained matmul activity.  The PE is idle anyway
            # while the first weights/x stream in (~0-11us), so burn that
            # window on junk N=64 matmuls into a scratch PSUM bank; the
            # real stream then starts at the full 2.4 GHz.
            wub = wup.tile([P, P], BF16)
            nc.gpsimd.memset(wub[:], 0)
            wups = wupp.tile([P, 64], F32)
            for _ in range(90):
                nc.tensor.matmul(wups[:], wub[:], wub[:, :64], start=True, stop=True)

            # Chunk-0 x rides at the FRONT of the gpsimd queue (it gates
            # the very first matmul; anywhere behind or beside the 16MB
            # weight stream it lands ~10us late because all DMA queues
            # share the 16 SDMA engines).
            xt0 = xp.tile([P, KH, chunks[0]], BF16, tag="xt")
            if chunks[0] == 512:
                nc.gpsimd.dma_start(xt0[:, :4], xda[:, 0, :4])
                nc.gpsimd.dma_start(xt0[:, 4:], xda[:, 0, 4:])
            else:
                nc.gpsimd.dma_start(xt0[:], xdb[:])

            # Weights resident in SBUF for the whole kernel.  The two lead
            # W1 slices ride the sync HWDGE ring right after chunk-0 x
            # (lowest latency); the bulk rides the gpsimd SWDGE path: the
            # sync ring carries x/y, and the scalar engine must stay free
            # for the GEMM1 activations (DMA issue ops block the issuing
            # engine on HWDGE FIFO space -- parking 18 weight DMAs on the
            # scalar queue delayed the first gelu to ~42us and stalled the
            # PE cold).  W1 first -- GEMM1 needs its lead slices
            # immediately; W2 isn't read until GEMM1 of chunk 0 finishes
            # (~50us in).
            w1t = w1p.tile([P, FT, KH, P], BF16)
            f0 = 0
            for si, g in enumerate(W1_SLICES):
                eng = nc.sync if si < 3 else nc.gpsimd
                eng.dma_start(w1t[:, f0 : f0 + g], w1d[:, f0 : f0 + g])
                f0 += g
            w2t = w2p.tile([P, FT, H], BF16)
            f0 = 0
            for g in W2_SLICES:
                nc.gpsimd.dma_start(w2t[:, f0 : f0 + g], w2d[:, f0 : f0 + g])
                f0 += g

            coff = 0
            for ci, nt in enumerate(chunks):
                if ci == 0:
                    xt = xt0
                else:
                    # later chunks have ~2 chunk-times of slack; ride the
                    # gpsimd queue BEHIND the weight bulk so they don't
                    # steal head HBM bandwidth from the W1 ramp.
                    xt = xp.tile([P, KH, nt], BF16, tag="xt")
                    if nt == 512:
                        nc.gpsimd.dma_start(xt[:], xda[:, ci])
                    else:
                        nc.gpsimd.dma_start(xt[:], xdb[:])

                # GEMM1: hT[f, :] = gelu(sum_k W1[k, f-tile].T @ xT[k, :])
                hq = hp.tile([P, FT, nt], BF16, tag="hq")
                for f in range(FT):
                    pt1 = ps1.tile([P, nt], F32, tag="pt1")
                    for k in range(KH):
                        nc.tensor.matmul(
                            pt1[:],
                            w1t[:, f, k, :],
                            xt[:, k, :],
                            start=(k == 0),
                            stop=(k == KH - 1),
                        )
                    bias = b1t[:, f : f + 1] if use_b1 else 0.0
                    nc.scalar.activation(hq[:, f, :], pt1[:], gelu, bias=bias)

                # GEMM2 (full F accumulation in PSUM):
                # Y[t-tile, hh] = sum_k2 hT[k2, t-tile].T @ W2[k2, hh]
                for t in range(-(-nt // P)):
                    tp = min(P, nt - t * P)  # partial tail t-tile
                    trow = coff // P + t
                    for hh in range(2):
                        pt2 = ps2.tile([P, 512], F32, tag="pt2")
                        for k2 in range(FT):
                            nc.tensor.matmul(
                                pt2[:tp],
                                hq[:, k2, t * P : t * P + tp],
                                w2t[:, k2, hh * 512 : (hh + 1) * 512],
                                start=(k2 == 0),
                                stop=(k2 == FT - 1),
                            )
                        ot = op.tile([P, 512], BF16, tag="ot")
                        nc.vector.tensor_copy(ot[:tp], pt2[:tp])
                        nc.sync.dma_start(
                            yd[:tp, trow, hh * 512 : (hh + 1) * 512], ot[:tp]
                        )
                coff += nt

    nc.compile()
    return nc


def _gelu_tanh(v):
    # jax.nn.gelu(approximate=True): 0.5x(1+tanh(sqrt(2/pi)(x+0.044715x^3)))
    return 0.5 * v * (1.0 + np.tanh(0.7978845608028654 * (v + 0.044715 * v**3)))


def _route(x2d, Wg):
    """Replicates reference router: softmax -> top-2 -> renormalize."""
    logits = x2d @ Wg  # [T, E] fp32
    m = logits.max(axis=-1, keepdims=True)
    p = np.exp(logits - m, dtype=np.float32)
    p /= p.sum(axis=-1, keepdims=True)
    # jax.lax.top_k: values descending, ties broken by lower index.
    order = np.argsort(-p, axis=-1, kind="stable")
    top_i = order[:, :TOP_K]  # [T, 2]
    top_p = np.take_along_axis(p, top_i, axis=-1)
    top_p = top_p / top_p.sum(axis=-1, keepdims=True)
    return top_i, top_p


def kernel(x, Wg, W1, b1, W2, b2):
    global LAST_RESULT
    x = np.ascontiguousarray(np.asarray(x, dtype=np.float32))
    Wg = np.ascontiguousarray(np.asarray(Wg, dtype=np.float32))
    W1 = np.ascontiguousarray(np.asarray(W1, dtype=np.float32))
    b1 = np.ascontiguousarray(np.asarray(b1, dtype=np.float32))
    W2 = np.ascontiguousarray(np.asarray(W2, dtype=np.float32))
    b2 = np.ascontiguousarray(np.asarray(b2, dtype=np.float32))

    x2d = x.reshape(T, H)
    top_i, top_p = _route(x2d, Wg)

    rows = [None] * E
    gval = [None] * E
    for e in range(E):
        r, slot = np.nonzero(top_i == e)
        rows[e] = r
        gval[e] = top_p[r, slot]

    # Expert capacity (factor 1.0): each core computes at most T*K/E =
    # 2048 token slots -- the perfectly balanced load, which keeps every
    # device chunk at the efficient 512 width and every GEMM2 t-tile at
    # full M=128.  The few overflow tokens of over-subscribed experts
    # (~1.8% of assignments for this routing) are evaluated in fp32
    # during the host-side combine below, exactly like the router and
    # gate application already are.
    cap = T * TOP_K // E
    c_max = max(len(r) for r in rows)
    c_pad = max(512, min(c_max, cap))
    chunks = _chunks_for(c_pad)
    n512 = sum(1 for c in chunks if c == 512)
    tail = chunks[-1] if chunks[-1] != 512 else None
    nrows = -(-c_pad // P)
    use_b1 = bool(np.any(b1))

    key = (c_pad, chunks, use_b1)
    if key not in _CACHE:
        _CACHE[key] = _build(c_pad, chunks, use_b1)
    nc = _CACHE[key]

    np_bf16 = mybir.dt.np(BF16)
    in_maps = []
    for e in range(E):
        cd = min(len(rows[e]), c_pad)
        xt = np.zeros((H, c_pad), np.float32)
        xt[:, :cd] = x2d[rows[e][:cd]].T
        # [P, KH, c_pad] view, then chunk-major repack
        xpkh = xt.reshape(KH, P, c_pad).transpose(1, 0, 2).astype(np_bf16)
        xa = np.ascontiguousarray(
            xpkh[:, :, : n512 * 512].reshape(P, KH, n512, 512).transpose(0, 2, 1, 3)
        )
        m = {
            "xda": xa,
            "w1d": np.ascontiguousarray(
                W1[e].reshape(KH, P, FT, P).transpose(1, 2, 0, 3).astype(np_bf16)
            ),
            "w2d": np.ascontiguousarray(
                W2[e].reshape(FT, P, H).transpose(1, 0, 2).astype(np_bf16)
            ),
        }
        if tail:
            m["xdb"] = np.ascontiguousarray(xpkh[:, :, n512 * 512 :])
        if use_b1:
            m["b1d"] = np.ascontiguousarray(b1[e].reshape(FT, P).T)
        in_maps.append(m)

    trace = os.environ.get("KERNEL_TRACE", "") == "1"
    res = run_bass_kernel_spmd(
        nc,
        in_maps,
        core_ids=list(range(N_CORES)),
        trace=trace,
        trace_cores=[0] if trace else None,
    )
    LAST_RESULT = res

    out = np.zeros((T, H), np.float32)
    for e in range(E):
        cd = min(len(rows[e]), c_pad)
        yt = res.results[e]["yd"].astype(np.float32)  # [P, nrows, H]
        y = yt.transpose(1, 0, 2).reshape(nrows * P, H)[:cd]
        out[rows[e][:cd]] += gval[e][:cd, None] * (y + b2[e][None, :])
        if len(rows[e]) > cd:  # capacity overflow: fp32 on host
            ro = rows[e][cd:]
            ho = _gelu_tanh(x2d[ro] @ W1[e] + b1[e][None, :])
            yo = ho @ W2[e] + b2[e][None, :]
            out[ro] += gval[e][cd:, None] * yo

    return out.reshape(B, S, H)
